# revision 1
# baseline (speedup 1.0000x reference)
"""Trainium2 Bass kernel for LocalAttentionLayer.

Problem: B=4, N=2048, H=8 heads, D=64, DM=512 (f32)
  q/k/v = x @ W{q,k,v}; sim = scale * q k^T (per head); mask_k/mask_q -> big_neg;
  softmax over keys; out = (attn @ v) @ Wo + bo.

Sharding (8 cores): core = 2*b + g -> batch b (4-way) x head-group g (2-way,
4 heads each).  Each core computes its batch's projections for its 4 heads,
full attention for those heads, and a partial output projection with its
256-row slice of Wo.  Host sums the two partials per batch, adds bo, and
overwrites masked-q rows (reference semantics: fully-masked rows degenerate
to uniform attention = mean over all v rows, computable on host as
(mean_j x) @ Wv @ Wo + bo).

Key optimizations over the naive layout:
  - Masked-position compaction: only kept q rows (Pq) and kept k rows (Pk)
    are shipped/computed; host gathers inputs and scatters outputs.  Pq/Pk
    are runtime values (q rounded up to 32, k to 128); one program is
    compiled per (Pq, Pk) and cached.
  - The attention inner loop is software-pipelined and ACT(Exp)-paced: sim
    j+2 is issued before pv j, each chunk's normalize is deferred until
    after the next chunk's first sims, and all non-attention PE work
    (k/q/v projections for later chunks, the hp1 projections, the output
    projection) is streamed through a work queue popped between sims so the
    Exp engine never waits at a phase boundary.
  - Softmax denominator rides along as a ones-column in v (col 64 of each
    68-wide head block), so P@V and the denominators come out of the same
    accumulation; all matmuls are bf16 (fp8 was measured too lossy: >1e-2).
  - PSUM-bank discipline: every matmul output stays inside one 2KB bank;
    the Exp uses strided 3-D APs to skip the inter-head alignment gap.
  - Copies and normalize run on DVE explicitly; ACT only does Exp.
  - Each input tensor loads as one wide multi-dim DMA (the HWDGE setup is
    a serial ~630ns/DMA resource), ordered by first use, with xq/xk split
    once so chunk-0 compute starts ~3us in.
"""

import sys

if "/opt/trn_rl_repo" not in sys.path:
    sys.path.insert(0, "/opt/trn_rl_repo")

from collections import deque

import os

import ml_dtypes
import numpy as np

SLACK_MARGIN = float(os.environ.get("K_SLACK", "274"))
LAG_LIMIT = int(os.environ.get("K_LAG", "28"))
PT_BUFS = int(os.environ.get("K_PTBUFS", "34"))

BF16 = np.dtype(ml_dtypes.bfloat16)

B, N, H, D = 4, 2048, 8, 64
DM = H * D  # 512
G = 2  # head-group split across cores
CG = DM // G  # 256 channels per group
HPG = H // G  # 4 heads per group
MASK_BIAS = -1.0e5
SHIFT = -4.0  # logit shift: keeps exp() comfortably in range without row max

_NC_CACHE = {}


def _build_nc(Pq, Pk):
    from contextlib import ExitStack

    import concourse.mybir as mybir
    import concourse.tile as tile
    from concourse import bacc
    from concourse.bass import ts

    f32 = mybir.dt.float32
    bf16 = mybir.dt.bfloat16
    EXP = mybir.ActivationFunctionType.Exp

    NJ = Pk // 128  # j-tiles
    # output i-tiles (last may be a partial tile: Pq is a multiple of 32)
    OT = []
    off = 0
    while off < Pq:
        OT.append((off, min(128, Pq - off)))
        off += 128
    NI = len(OT)
    # attention i-chunks: full 512-wide (exact PSUM bank per head) with a
    # small remainder chunk LAST, so the epilogue (normalize + out-proj of
    # the final chunk) is as short as possible
    CHS = [512] * (Pq // 512) + ([Pq % 512] if Pq % 512 else [])
    COFF = [sum(CHS[:i]) for i in range(len(CHS))]
    ICK = Pk // 4  # k-projection chunk

    nc = bacc.Bacc(None, target_bir_lowering=False, debug=False)

    with tile.TileContext(nc) as tc, ExitStack() as ctx:
        dram = ctx.enter_context(tc.tile_pool(name="dram", bufs=1, space="DRAM"))
        const = ctx.enter_context(tc.tile_pool(name="const", bufs=1))
        ptp = ctx.enter_context(tc.tile_pool(name="ptp", bufs=PT_BUFS))
        fop = ctx.enter_context(tc.tile_pool(name="fop", bufs=4))
        rrp = ctx.enter_context(tc.tile_pool(name="rrp", bufs=2))
        psim = ctx.enter_context(tc.tile_pool(name="psim", bufs=2, space="PSUM"))
        ppv = ctx.enter_context(tc.tile_pool(name="ppv", bufs=1, space="PSUM"))
        pfo = ctx.enter_context(tc.tile_pool(name="pfo", bufs=2, space="PSUM"))

        # ---- DRAM I/O ----
        xqT_d = dram.tile([DM, Pq], bf16, kind="ExternalInput", name="xqT", uniquify=False)
        xkT_d = dram.tile([DM, Pk], bf16, kind="ExternalInput", name="xkT", uniquify=False)
        wq_d = dram.tile([DM, CG], bf16, kind="ExternalInput", name="wq", uniquify=False)
        wk_d = dram.tile([DM, CG], bf16, kind="ExternalInput", name="wk", uniquify=False)
        # wv pre-arranged on host: [512, HPG*65]; per head h cols h*65..h*65+63
        # are Wv columns, col h*65+64 is the (zero) ones-column slot
        wva_d = dram.tile([DM, HPG * 65], bf16, kind="ExternalInput", name="wva", uniquify=False)
        wo_d = dram.tile([CG, DM], bf16, kind="ExternalInput", name="wo", uniquify=False)
        bk_d = dram.tile([128, NJ], f32, kind="ExternalInput", name="bk", uniquify=False)
        out_d = dram.tile([Pq, DM], f32, kind="ExternalOutput", name="out", uniquify=False)

        # ---- SBUF persistents ----
        # The HWDGE + DMA engines are a serial resource (~630ns setup per
        # DMA), so each tensor loads as ONE wide DMA ([128, slices, cols]
        # APs), ordered by first use; xq/xk split once so chunk-0 arrives
        # early and compute starts ~3us in.
        xq_r = xqT_d.rearrange("(s p) i -> p s i", s=4, p=128)
        xk_r = xkT_d.rearrange("(s p) i -> p s i", s=4, p=128)
        wq_sb = const.tile([128, 4, CG], bf16, name="wq_sb")
        nc.sync.dma_start(out=wq_sb[:, :, :], in_=wq_d.rearrange("(s p) c -> p s c", s=4, p=128))
        xqT_sb = const.tile([128, 4, Pq], bf16, name="xqT_sb")
        nc.sync.dma_start(out=xqT_sb[:, :, 0 : CHS[0]], in_=xq_r[:, :, 0 : CHS[0]])
        wk_sb = const.tile([128, 4, CG], bf16, name="wk_sb")
        nc.sync.dma_start(out=wk_sb[:, :, :], in_=wk_d.rearrange("(s p) c -> p s c", s=4, p=128))
        bk_sb = const.tile_from(bk_d[:, :], name="bks")
        xkT_sb = const.tile([128, 4, Pk], bf16, name="xkT_sb")
        nc.sync.dma_start(out=xkT_sb[:, :, 0:ICK], in_=xk_r[:, :, 0:ICK])
        wva_sb = const.tile([128, 4, HPG * 65], bf16, name="wva_sb")
        nc.sync.dma_start(out=wva_sb[:, :, :], in_=wva_d.rearrange("(s p) c -> p s c", s=4, p=128))
        nc.sync.dma_start(out=xkT_sb[:, :, ICK:Pk], in_=xk_r[:, :, ICK:Pk])
        nc.sync.dma_start(out=xqT_sb[:, :, CHS[0] : Pq], in_=xq_r[:, :, CHS[0] : Pq])
        wo_sb = const.tile([128, 2, DM], bf16, name="wo_sb")
        nc.sync.dma_start(out=wo_sb[:, :, :], in_=wo_d.rearrange("(s p) c -> p s c", s=2, p=128))

        ones_row = const.tile([1, Pk], bf16, name="ones_row")
        nc.vector.memset(ones_row[:, :], 1.0)
        wv5 = const.tile([1, HPG * 65], bf16, name="wv5")
        nc.vector.memset(wv5[:, :], 0.0)
        for h in range(HPG):
            nc.vector.memset(wv5[:, h * 65 + 64 : h * 65 + 65], 1.0)
        ones64 = const.tile([1, 64], bf16, name="ones64")
        nc.vector.memset(ones64[:, :], 1.0)

        qT_sb = [const.tile([128, Pq], bf16, name=f"qT{hp}") for hp in range(2)]
        kT_sb = [const.tile([128, Pk], bf16, name=f"kT{hp}") for hp in range(2)]
        aT_sb = [const.tile([128, Pq], bf16, name=f"aT{hp}") for hp in range(2)]
        # va: per j-tile [128, HPG*65] bf16: 4 heads x (64 v-cols + ones col)
        va_sb = [const.tile([128, HPG * 65], bf16, name=f"va{j}") for j in range(NJ)]

        # ---- projection / output helpers ----
        def qk_proj_group(w_sb, x_sb, dst, hp, off, width):
            """One chunk of a q/k projection: dst[:, off:off+width]."""
            ps = pfo.tile([128, 512], f32, tag="fo", name="qk_ps")
            for k in range(4):
                nc.tensor.matmul(
                    ps[:, 0:width],
                    w_sb[:, k, hp * 128 : (hp + 1) * 128],
                    x_sb[:, k, off : off + width],
                    start=(k == 0),
                    stop=(k == 3),
                )
            nc.vector.tensor_copy(dst[:, off : off + width], ps[:, 0:width])

        def push_qk(w_sb, x_sb, dst, hp, off, width):
            push(QK_NS, lambda: qk_proj_group(w_sb, x_sb, dst, hp, off, width))

        def v_proj(j):
            v_ps = pfo.tile([128, HPG * 65], f32, tag="fo", name="v_ps")
            for k in range(4):
                nc.tensor.matmul(
                    v_ps[:, :],
                    xkT_sb[:, k, ts(j, 128)],
                    wva_sb[:, k, :],
                    start=(k == 0),
                    stop=False,
                )
            nc.tensor.matmul(
                v_ps[:, :],
                ones_row[:, ts(j, 128)],
                wv5[:, :],
                start=False,
                stop=True,
            )
            nc.vector.tensor_copy(va_sb[j][:, :], v_ps[:, :])

        def out_proj(it):
            io, isz = OT[it]
            fo = pfo.tile([128, 512], f32, tag="fo", name="fo_ps")
            for c in range(2):
                nc.tensor.matmul(
                    fo[0:isz, :],
                    aT_sb[c][:, io : io + isz],
                    wo_sb[:, c, :],
                    start=(c == 0),
                    stop=(c == 1),
                )
            fo_sb = fop.tile([128, 512], f32, tag="fos", name="fo_sb")
            nc.vector.tensor_copy(fo_sb[0:isz, :], fo[0:isz, :])
            nc.sync.dma_start(out=out_d[io : io + isz, :], in_=fo_sb[0:isz, :])

        # ---- attention: globally slot-scheduled ----
        # PE is the binding engine (~107us vs ACT ~98us).  Each j slot emits
        # its sim matmuls (the only fixed-cadence PE work, ~374ns) and then
        # pops deferred PE units (pv accumulations, v/q/k projections, the
        # normalizes, output projection) from a FIFO under a per-slot cost
        # budget, so PE stays packed while sims always lead the Exp stream.
        work_q = deque()  # (est_cost_ns, emit_fn, is_pv)
        budget = [0.0]
        pv_lag = [0]  # un-popped pv units; must stay under the pt ring depth
        PV_LAG_LIMIT = LAG_LIMIT

        def push(cost, fn, is_pv=False):
            work_q.append((cost, fn, is_pv))
            if is_pv:
                pv_lag[0] += 1

        def _pop_one():
            cost, fn, is_pv = work_q.popleft()
            budget[0] -= cost
            if is_pv:
                pv_lag[0] -= 1
            fn()

        def pops():
            while work_q and work_q[0][0] <= budget[0]:
                _pop_one()
            # a pv backlog deeper than the pt ring would deadlock the
            # in-order engines: force-drain ahead of budget
            while pv_lag[0] > PV_LAG_LIMIT:
                _pop_one()


        V_NS = 5 * HPG * 65 * 0.42
        QK_NS = 4 * 512 * 0.42
        OUT_NS = (2 * DM + DM) * 0.42

        done_tiles = [0]

        def attention(hp, ci):
            i0, ic = COFF[ci], CHS[ci]
            slot_slack = 2 * ic * 0.4167 + SLACK_MARGIN
            if (hp, ci) == (0, 0):
                slot_slack += float(os.environ.get("K_SLACK0", "0"))
            if hp == 1:
                slot_slack += float(os.environ.get("K_SLACK1", "0"))
            pv = [ppv.tile([65, ic], f32, tag=f"pv{h}", name=f"pv{h}") for h in range(2)]

            def emit_sim(j):
                # head blocks bank-aligned at h*512: a matmul output must not
                # cross a PSUM bank boundary
                sm = psim.tile([128, 1024], f32, tag="sim", name="sm")
                for h in range(2):
                    hs = slice(h * 64, (h + 1) * 64)
                    nc.tensor.matmul(
                        sm[:, h * 512 : h * 512 + ic],
                        kT_sb[hp][hs, ts(j, 128)],
                        qT_sb[hp][hs, i0 : i0 + ic],
                        start=True,
                        stop=True,
                    )
                ptv = ptp.tile([128, 2 * 512], bf16, tag="pt", name="pt")
                sm_v = sm.rearrange("p (h c) -> p h c", h=2, c=512)[:, :, 0:ic]
                pt_v = ptv.rearrange("p (h c) -> p h c", h=2, c=512)[:, :, 0:ic]
                nc.scalar.activation(pt_v, sm_v, EXP, bias=bk_sb[:, j : j + 1], scale=1.0)
                return ptv

            def mk_pv(j, ptv):
                def f():
                    for h in range(2):
                        hb = (hp * 2 + h) * 65
                        nc.tensor.matmul(
                            pv[h][:, :],
                            va_sb[j][:, hb : hb + 65],
                            ptv[:, h * 512 : h * 512 + ic],
                            start=(j == 0),
                            stop=(j == NJ - 1),
                        )
                return f

            pts = {0: emit_sim(0)}
            for j in range(NJ):
                if j + 1 < NJ:
                    pts[j + 1] = emit_sim(j + 1)
                if (hp, ci) == (0, 0) and j + 2 < NJ:
                    push(V_NS, lambda j2=j + 2: v_proj(j2))
                push(2 * ic * 0.42, mk_pv(j, pts.pop(j)), is_pv=True)
                budget[0] += slot_slack
                pops()

            def normalize():
                rr = [rrp.tile([1, ic], bf16, tag=f"rr{h}", name=f"rr{h}") for h in range(2)]
                with nc.allow_low_precision(reason="1/denom in bf16; ~2^-9 relative"):
                    for h in range(2):
                        nc.vector.reciprocal(rr[h][:, :], pv[h][64:65, :])
                rb = psim.tile([128, ic], f32, tag="sim", name="rb")
                for h in range(2):
                    nc.tensor.matmul(
                        rb[h * 64 : (h + 1) * 64, :],
                        ones64[:, :],
                        rr[h][:, :],
                        start=True,
                        stop=True,
                    )
                rb_sb = rrp.tile([128, ic], bf16, tag="rbs", name="rb_sb")
                nc.vector.tensor_copy(rb_sb[:, :], rb[:, :])
                for h in range(2):
                    nc.vector.tensor_mul(
                        aT_sb[hp][h * 64 : (h + 1) * 64, i0 : i0 + ic],
                        pv[h][0:64, :],
                        rb_sb[h * 64 : (h + 1) * 64, :],
                    )
                if hp == 1:
                    upto = sum(1 for io, isz in OT if io + isz <= i0 + ic)
                    while done_tiles[0] < upto:
                        it = done_tiles[0]
                        push(OUT_NS, lambda it=it: out_proj(it))
                        done_tiles[0] += 1

            push(2 * ic * 0.42, normalize)

        # ---- program ----
        qk_proj_group(wq_sb, xqT_sb, qT_sb[0], 0, 0, CHS[0])
        qk_proj_group(wk_sb, xkT_sb, kT_sb[0], 0, 0, ICK)
        v_proj(0)
        v_proj(1)

        # seed deferred work: k/q chunks first (hard deadlines: k for this
        # chunk's own sims, q before chunk 1), v next (needed only when the
        # corresponding pv pops, well after slot j)
        for ch in range(1, 4):
            push_qk(wk_sb, xkT_sb, kT_sb[0], 0, ch * ICK, ICK)
        for ci in range(1, len(CHS)):
            push_qk(wq_sb, xqT_sb, qT_sb[0], 0, COFF[ci], CHS[ci])
        for hp in range(2):
            for ci in range(len(CHS)):
                if (hp, ci) == (0, 1):
                    # hp1 projections: pushed here so they pop before hp1
                    for c2 in range(len(CHS)):
                        push_qk(wq_sb, xqT_sb, qT_sb[1], 1, COFF[c2], CHS[c2])
                    for c2 in range(4):
                        push_qk(wk_sb, xkT_sb, kT_sb[1], 1, c2 * ICK, ICK)
                attention(hp, ci)
        budget[0] = 1e12
        pops()
        while done_tiles[0] < NI:
            out_proj(done_tiles[0])
            done_tiles[0] += 1

    nc.compile()
    return nc


def _get_nc(Pq=1696, Pk=1664):
    key = (Pq, Pk)
    if key not in _NC_CACHE:
        _NC_CACHE[key] = _build_nc(Pq, Pk)
    return _NC_CACHE[key]


def _roundup(n, m):
    return ((n + m - 1) // m) * m


def kernel(x, Wq, Wk, Wv, Wo, bo, mask_k, mask_q):
    from concourse import bass_utils

    x = np.asarray(x, np.float32)
    Wq = np.asarray(Wq, np.float32)
    Wk = np.asarray(Wk, np.float32)
    Wv = np.asarray(Wv, np.float32)
    Wo = np.asarray(Wo, np.float32)
    bo = np.asarray(bo, np.float32)
    mask_k = np.asarray(mask_k)
    mask_q = np.asarray(mask_q)

    qidx = [np.nonzero(mask_q[b])[0] for b in range(B)]
    kidx = [np.nonzero(mask_k[b])[0] for b in range(B)]
    # compaction pads; >=1024 keeps the chunk layout simple
    Pq = max(_roundup(max(len(i) for i in qidx), 32), 1024)
    Pk = max(_roundup(max(len(i) for i in kidx), 128), 1024)
    NJ = Pk // 128

    nc = _get_nc(Pq, Pk)
    scale = float(D) ** -0.5

    in_maps = []
    for core in range(8):
        b, g = core // 2, core % 2
        cs = slice(g * CG, (g + 1) * CG)
        qi, ki = qidx[b], kidx[b]
        xq = np.zeros((Pq, DM), np.float32)
        xq[: len(qi)] = x[b][qi]
        xk = np.zeros((Pk, DM), np.float32)
        xk[: len(ki)] = x[b][ki]
        wva = np.zeros((DM, HPG * 65), np.float32)
        for h in range(HPG):
            wva[:, h * 65 : h * 65 + 64] = Wv[:, g * CG + h * 64 : g * CG + (h + 1) * 64]
        bk = np.full((Pk,), MASK_BIAS, np.float32)
        bk[: len(ki)] = SHIFT
        in_maps.append(
            {
                "xqT": np.ascontiguousarray(xq.T).astype(BF16),
                "xkT": np.ascontiguousarray(xk.T).astype(BF16),
                "wq": np.ascontiguousarray(Wq[:, cs] * scale).astype(BF16),
                "wk": np.ascontiguousarray(Wk[:, cs]).astype(BF16),
                "wva": wva.astype(BF16),
                "wo": np.ascontiguousarray(Wo[cs, :]).astype(BF16),
                "bk": np.ascontiguousarray(bk.reshape(NJ, 128).T),
            }
        )

    global _LAST_IN_MAPS, _LAST_NC
    _LAST_IN_MAPS = in_maps
    _LAST_NC = nc
    res = bass_utils.run_bass_kernel_spmd(nc, in_maps, core_ids=list(range(8)))
    outs = res.results

    out = np.empty((B, N, DM), np.float32)
    for b in range(B):
        o = outs[2 * b]["out"] + outs[2 * b + 1]["out"]
        full = np.empty((N, DM), np.float32)
        full[qidx[b]] = o[: len(qidx[b])] + bo[None, :]
        # reference semantics for fully-masked query rows: uniform attention
        uf = (x[b].mean(0) @ Wv) @ Wo + bo
        full[~mask_q[b]] = uf
        out[b] = full
    return out

